# revision 1
# baseline (speedup 1.0000x reference)
"""Masked attention (B=16, S=1024, H=1024) on 8 TRN2 NeuronCores.

Strategy: pure data-parallel over batch — 2 batches per core, no collectives.
Per batch the core computes Q/K/V projections, masked softmax attention, and
both outputs (attended values + attention weights).

Math (per batch, X = input[b] of shape [S, H]):
  qT = (Wq/sqrt(H)) @ X.T + bq/sqrt(H)    -> [H, S]   (kept transposed)
  kT = Wk @ X.T + bk                      -> [H, S]
  v  = X @ Wv.T + bv                      -> [S, H]   (kept untransposed)
  s  = qT.T @ kT + maskbias               -> [S, S]   (maskbias = -1e9 where mask==0)
  e  = exp(s - rowmax); P = e / rowsum
  att = (e.T.T @ v) * (1/rowsum)          -> e transposed on the PE for the PV matmul

All TensorEngine operands are bf16 (inputs pre-cast on host), accumulation is
f32 in PSUM, softmax statistics are f32.  Measured end-to-end rel err vs the
f32 reference is ~5e-3.

Host-side prep per core c: x batches [2c, 2c+1] transposed to [2, H, S] bf16;
weights transposed (and Wq, bq pre-scaled by 1/sqrt(H)) to bf16; mask turned
into an additive f32 bias replicated over 128 partitions.
"""
import numpy as np
import ml_dtypes

import concourse.bass as bass
import concourse.mybir as mybir
from concourse import bacc
from concourse.tile import TileContext
from concourse.bass_utils import run_bass_kernel_spmd
from concourse.masks import make_identity

B, S, H = 16, 1024, 1024
P = 128
NCORES = 8
B_LOC = B // NCORES          # batches per core
KT = H // P                  # 8 contraction tiles
RT = S // P                  # 8 row blocks
NFREE = 512                  # matmul moving free dim (one PSUM bank)
NCH = S // NFREE             # 2 free-dim chunks
BF16 = mybir.dt.bfloat16
F32 = mybir.dt.float32

_BUILD_CACHE = {}


def build():
    if "nc" in _BUILD_CACHE:
        return _BUILD_CACHE["nc"]
    nc = bacc.Bacc()

    xT = nc.declare_dram_parameter("xT", [B_LOC, H, S], BF16, isOutput=False)
    wqT = nc.declare_dram_parameter("wqT", [H, H], BF16, isOutput=False)
    wkT = nc.declare_dram_parameter("wkT", [H, H], BF16, isOutput=False)
    wvT = nc.declare_dram_parameter("wvT", [H, H], BF16, isOutput=False)
    bqp = nc.declare_dram_parameter("bqp", [P, KT], F32, isOutput=False)
    bkp = nc.declare_dram_parameter("bkp", [P, KT], F32, isOutput=False)
    bvr = nc.declare_dram_parameter("bvr", [P, H], F32, isOutput=False)
    mkb = nc.declare_dram_parameter("mkb", [B_LOC, P, S], F32, isOutput=False)
    att = nc.declare_dram_parameter("att", [B_LOC, S, H], F32, isOutput=True)
    attw = nc.declare_dram_parameter("attw", [B_LOC, S, S], F32, isOutput=True)

    with TileContext(nc) as tc:
        with (
            tc.tile_pool(name="const", bufs=1) as constp,
            tc.tile_pool(name="wpool", bufs=1) as wpool,
            tc.tile_pool(name="xpool", bufs=2) as xpool,
            tc.tile_pool(name="qkv", bufs=1) as qkvp,
            tc.tile_pool(name="soft", bufs=2) as soft,
            tc.tile_pool(name="stats", bufs=3) as stats,
            tc.tile_pool(name="psmm", bufs=6, space="PSUM") as psmm,
            tc.tile_pool(name="pstr", bufs=2, space="PSUM") as pstr,
        ):
            ident = constp.tile([P, P], BF16)
            make_identity(nc, ident)
            bq_t = constp.tile([P, KT], F32)
            bk_t = constp.tile([P, KT], F32)
            bv_t = constp.tile([P, H], F32)
            nc.sync.dma_start(out=bq_t, in_=bqp[:, :])
            nc.sync.dma_start(out=bk_t, in_=bkp[:, :])
            nc.sync.dma_start(out=bv_t, in_=bvr[:, :])

            wq_t = wpool.tile([P, KT, H], BF16)
            wk_t = wpool.tile([P, KT, H], BF16)
            wv_t = wpool.tile([P, KT, H], BF16)
            for w_dram, w_tile in ((wqT, wq_t), (wkT, wk_t), (wvT, wv_t)):
                nc.sync.dma_start(out=w_tile, in_=w_dram[:].rearrange("(t p) o -> p t o", p=P))

            for b in range(B_LOC):
                xT_t = xpool.tile([P, KT, S], BF16, name="xT_t", tag="xT")
                nc.sync.dma_start(out=xT_t, in_=xT[b].rearrange("(t p) s -> p t s", p=P))
                mkb_t = xpool.tile([P, S], F32, name="mkb_t", tag="mkb")
                nc.sync.dma_start(out=mkb_t, in_=mkb[b])

                qT_t = qkvp.tile([P, KT, S], BF16, name="qT_t", tag="qT")
                kT_t = qkvp.tile([P, KT, S], BF16, name="kT_t", tag="kT")
                v_t = qkvp.tile([P, KT, H], BF16, name="v_t", tag="v")

                # ---- projections ----
                # qT[o, s] / kT[o, s]: lhsT = w?T[:, kt, o_blk], rhs = xT[:, kt, s_chunk]
                for ot in range(KT):
                    for ch in range(NCH):
                        sl = slice(ch * NFREE, (ch + 1) * NFREE)
                        ps_q = psmm.tile([P, NFREE], F32, name="ps_q", tag="mm")
                        for kt in range(KT):
                            nc.tensor.matmul(ps_q, wq_t[:, kt, ot * P:(ot + 1) * P],
                                             xT_t[:, kt, sl], start=(kt == 0), stop=(kt == KT - 1))
                        nc.vector.tensor_scalar_add(qT_t[:, ot, sl], ps_q, bq_t[:, ot:ot + 1])
                        ps_k = psmm.tile([P, NFREE], F32, name="ps_k", tag="mm")
                        for kt in range(KT):
                            nc.tensor.matmul(ps_k, wk_t[:, kt, ot * P:(ot + 1) * P],
                                             xT_t[:, kt, sl], start=(kt == 0), stop=(kt == KT - 1))
                        nc.vector.tensor_scalar_add(kT_t[:, ot, sl], ps_k, bk_t[:, ot:ot + 1])
                # v[s, o]: lhsT = xT[:, kt, s_blk], rhs = wvT[:, kt, o_chunk]
                for st in range(RT):
                    for ch in range(NCH):
                        sl = slice(ch * NFREE, (ch + 1) * NFREE)
                        ps_v = psmm.tile([P, NFREE], F32, name="ps_v", tag="mm")
                        for kt in range(KT):
                            nc.tensor.matmul(ps_v, xT_t[:, kt, st * P:(st + 1) * P],
                                             wv_t[:, kt, sl], start=(kt == 0), stop=(kt == KT - 1))
                        nc.vector.tensor_tensor(out=v_t[:, st, sl], in0=ps_v, in1=bv_t[:, sl],
                                                op=mybir.AluOpType.add)

                # ---- attention, one row block (128 queries) at a time ----
                for r in range(RT):
                    sc_t = soft.tile([P, S], F32, name="sc_t", tag="sc")
                    for ch in range(NCH):
                        sl = slice(ch * NFREE, (ch + 1) * NFREE)
                        ps_s = psmm.tile([P, NFREE], F32, name="ps_s", tag="mm")
                        for ot in range(KT):
                            nc.tensor.matmul(ps_s, qT_t[:, ot, r * P:(r + 1) * P],
                                             kT_t[:, ot, sl], start=(ot == 0), stop=(ot == KT - 1))
                        nc.vector.tensor_tensor(out=sc_t[:, sl], in0=ps_s, in1=mkb_t[:, sl],
                                                op=mybir.AluOpType.add)

                    negmax = stats.tile([P, 1], F32, name="negmax", tag="negmax")
                    nc.vector.reduce_max(out=negmax, in_=sc_t, axis=mybir.AxisListType.X, negate=True)
                    e_t = soft.tile([P, S], BF16, name="e_t", tag="e")
                    rowsum = stats.tile([P, 1], F32, name="rowsum", tag="rowsum")
                    nc.scalar.activation(out=e_t, in_=sc_t, func=mybir.ActivationFunctionType.Exp,
                                         bias=negmax, scale=1.0, accum_out=rowsum)
                    recip = stats.tile([P, 1], F32, name="recip", tag="recip")
                    nc.vector.reciprocal(out=recip, in_=rowsum)

                    # attention-weights output: P = e * recip
                    p_t = soft.tile([P, S], F32, name="p_t", tag="p")
                    nc.vector.tensor_scalar_mul(p_t, e_t, recip)
                    nc.sync.dma_start(out=attw[b, r * P:(r + 1) * P, :], in_=p_t)

                    # transpose e on the PE: eT[j, i] tiles
                    eT_t = soft.tile([P, RT, P], BF16, name="eT_t", tag="eT")
                    for jt in range(RT):
                        ps_t = pstr.tile([P, P], BF16, name="ps_t", tag="tr")
                        nc.tensor.transpose(ps_t, e_t[:, jt * P:(jt + 1) * P], ident)
                        nc.scalar.activation(out=eT_t[:, jt], in_=ps_t,
                                             func=mybir.ActivationFunctionType.Copy)

                    # att[i, h] = sum_j e[i, j] v[j, h], normalized by recip
                    at_t = soft.tile([P, H], F32, name="at_t", tag="at")
                    for ch in range(NCH):
                        sl = slice(ch * NFREE, (ch + 1) * NFREE)
                        ps_a = psmm.tile([P, NFREE], F32, name="ps_a", tag="mm")
                        for jt in range(RT):
                            nc.tensor.matmul(ps_a, eT_t[:, jt], v_t[:, jt, sl],
                                             start=(jt == 0), stop=(jt == RT - 1))
                        nc.vector.tensor_scalar_mul(at_t[:, sl], ps_a, recip)
                    nc.sync.dma_start(out=att[b, r * P:(r + 1) * P, :], in_=at_t)

    nc.finalize()
    _BUILD_CACHE["nc"] = nc
    return nc


def _bf16(x):
    return np.ascontiguousarray(x.astype(ml_dtypes.bfloat16))


def kernel(input, mask, Wq, bq, Wk, bk, Wv, bv):
    input = np.asarray(input, dtype=np.float32)
    mask = np.asarray(mask)
    scale = np.float32(1.0 / np.sqrt(H))

    wqT = _bf16(Wq.T * scale)
    wkT = _bf16(np.asarray(Wk).T)
    wvT = _bf16(np.asarray(Wv).T)
    bqp = np.ascontiguousarray((np.asarray(bq, dtype=np.float32) * scale).reshape(KT, P).T)
    bkp = np.ascontiguousarray(np.asarray(bk, dtype=np.float32).reshape(KT, P).T)
    bvr = np.ascontiguousarray(np.broadcast_to(np.asarray(bv, dtype=np.float32), (P, H)))

    in_maps = []
    for c in range(NCORES):
        xb = input[c * B_LOC:(c + 1) * B_LOC]                       # [B_LOC, S, H]
        xT = _bf16(xb.transpose(0, 2, 1))                           # [B_LOC, H, S]
        mb = np.where(mask[c * B_LOC:(c + 1) * B_LOC, 0, 0, :] == 0,
                      np.float32(-1e9), np.float32(0.0))            # [B_LOC, S]
        mkb = np.ascontiguousarray(np.broadcast_to(mb[:, None, :], (B_LOC, P, S)),
                                   dtype=np.float32)
        in_maps.append({
            "xT": xT, "wqT": wqT, "wkT": wkT, "wvT": wvT,
            "bqp": bqp, "bkp": bkp, "bvr": bvr, "mkb": mkb,
        })

    nc = build()
    res = run_bass_kernel_spmd(nc, in_maps, core_ids=list(range(NCORES)))
    att = np.concatenate([res.results[c]["att"] for c in range(NCORES)], axis=0)
    attw = np.concatenate([res.results[c]["attw"] for c in range(NCORES)], axis=0)
    return att, attw


# revision 3
# speedup vs baseline: 1.0763x; 1.0763x over previous
"""Masked attention (B=16, S=1024, H=1024) on 8 TRN2 NeuronCores.

Strategy: pure data-parallel over batch — 2 batches per core, no collectives.
Per batch the core computes Q/K/V projections, masked softmax attention, and
both outputs (attended values + attention weights).

Math (per batch, X = input[b] of shape [S, H]):
  qT = (Wq/sqrt(H)) @ X.T + bq/sqrt(H)    -> [H, S]   (kept transposed)
  kT = Wk @ X.T + bk                      -> [H, S]
  v  = X @ Wv.T + bv                      -> [S, H]   (kept untransposed)
  s  = qT.T @ kT + maskbias               -> [S, S]   (maskbias = -1e9 where mask==0)
  e  = exp(s - rowmax); P = e / rowsum
  att = (e.T.T @ v) * (1/rowsum)          -> e transposed on the PE for the PV matmul

All TensorEngine operands are bf16 (inputs pre-cast on host), accumulation is
f32 in PSUM, softmax statistics are f32.  Measured end-to-end rel err vs the
f32 reference is ~5e-3.

Host-side prep per core c: x batches [2c, 2c+1] transposed to [2, H, S] bf16;
weights transposed (and Wq, bq pre-scaled by 1/sqrt(H)) to bf16; mask turned
into an additive f32 bias replicated over 128 partitions.
"""
import numpy as np
import ml_dtypes

import concourse.bass as bass
import concourse.mybir as mybir
from concourse import bacc
from concourse.tile import TileContext
from concourse.bass_utils import run_bass_kernel_spmd
from concourse.masks import make_identity

B, S, H = 16, 1024, 1024
P = 128
NCORES = 8
B_LOC = B // NCORES          # batches per core
KT = H // P                  # 8 contraction tiles
RT = S // P                  # 8 row blocks
NFREE = 512                  # matmul moving free dim (one PSUM bank)
NCH = S // NFREE             # 2 free-dim chunks
BF16 = mybir.dt.bfloat16
F32 = mybir.dt.float32

_BUILD_CACHE = {}


def build():
    if "nc" in _BUILD_CACHE:
        return _BUILD_CACHE["nc"]
    nc = bacc.Bacc()

    xT = nc.declare_dram_parameter("xT", [B_LOC, H, S], BF16, isOutput=False)
    wqT = nc.declare_dram_parameter("wqT", [H, H], BF16, isOutput=False)
    wkT = nc.declare_dram_parameter("wkT", [H, H], BF16, isOutput=False)
    wvT = nc.declare_dram_parameter("wvT", [H, H], BF16, isOutput=False)
    bqp = nc.declare_dram_parameter("bqp", [P, KT], F32, isOutput=False)
    bkp = nc.declare_dram_parameter("bkp", [P, KT], F32, isOutput=False)
    bvr = nc.declare_dram_parameter("bvr", [P, H], F32, isOutput=False)
    mkb = nc.declare_dram_parameter("mkb", [B_LOC, P, S], F32, isOutput=False)
    att = nc.declare_dram_parameter("att", [B_LOC, S, H], F32, isOutput=True)
    attw = nc.declare_dram_parameter("attw", [B_LOC, S, S], F32, isOutput=True)

    with TileContext(nc) as tc:
        with (
            tc.tile_pool(name="const", bufs=1) as constp,
            tc.tile_pool(name="wpool", bufs=1) as wpool,
            tc.tile_pool(name="xpool", bufs=2) as xpool,
            tc.tile_pool(name="qkv", bufs=1) as qkvp,
            tc.tile_pool(name="soft", bufs=2) as soft,
            tc.tile_pool(name="stats", bufs=3) as stats,
            tc.tile_pool(name="psmm", bufs=6, space="PSUM") as psmm,
            tc.tile_pool(name="pstr", bufs=2, space="PSUM") as pstr,
        ):
            ident = constp.tile([P, P], BF16)
            make_identity(nc, ident)
            bq_t = constp.tile([P, KT], F32)
            bk_t = constp.tile([P, KT], F32)
            bv_t = constp.tile([P, H], F32)
            nc.sync.dma_start(out=bq_t, in_=bqp[:, :])
            nc.sync.dma_start(out=bk_t, in_=bkp[:, :])
            nc.sync.dma_start(out=bv_t, in_=bvr[:, :])

            # Weights split into per-output-block DMAs so the first projection
            # group's dependencies resolve after ~0.75 MB instead of ~8 MB.
            wq_t = wpool.tile([P, KT, H], BF16)
            wk_t = wpool.tile([P, KT, H], BF16)
            wv_t = wpool.tile([P, KT, H], BF16)
            wq_r = wqT[:].rearrange("(t p) o -> p t o", p=P)
            wk_r = wkT[:].rearrange("(t p) o -> p t o", p=P)
            wv_r = wvT[:].rearrange("(t p) o -> p t o", p=P)
            xT0_t = xpool.tile([P, KT, S], BF16, name="xT0_t", tag="xT")
            xT0_r = xT[0].rearrange("(t p) s -> p t s", p=P)
            nc.sync.dma_start(out=wq_t[:, :, 0:P], in_=wq_r[:, :, 0:P])
            for ch in range(NCH):
                sl = slice(ch * NFREE, (ch + 1) * NFREE)
                nc.sync.dma_start(out=xT0_t[:, :, sl], in_=xT0_r[:, :, sl])
            for ot in range(1, KT):
                sl = slice(ot * P, (ot + 1) * P)
                nc.sync.dma_start(out=wq_t[:, :, sl], in_=wq_r[:, :, sl])
            for ot in range(KT):
                sl = slice(ot * P, (ot + 1) * P)
                nc.sync.dma_start(out=wk_t[:, :, sl], in_=wk_r[:, :, sl])
            for ch in range(NCH):
                sl = slice(ch * NFREE, (ch + 1) * NFREE)
                nc.sync.dma_start(out=wv_t[:, :, sl], in_=wv_r[:, :, sl])

            for b in range(B_LOC):
                if b == 0:
                    xT_t = xT0_t
                else:
                    xT_t = xpool.tile([P, KT, S], BF16, name="xT_t", tag="xT")
                    nc.sync.dma_start(out=xT_t, in_=xT[b].rearrange("(t p) s -> p t s", p=P))
                mkb_t = xpool.tile([P, S], F32, name="mkb_t", tag="mkb")
                nc.sync.dma_start(out=mkb_t, in_=mkb[b])

                qT_t = qkvp.tile([P, KT, S], BF16, name="qT_t", tag="qT")
                kT_t = qkvp.tile([P, KT, S], BF16, name="kT_t", tag="kT")
                v_t = qkvp.tile([P, KT, H], BF16, name="v_t", tag="v")

                # ---- projections ----
                # qT[o, s] / kT[o, s]: lhsT = w?T[:, kt, o_blk], rhs = xT[:, kt, s_chunk]
                for ot in range(KT):
                    for ch in range(NCH):
                        sl = slice(ch * NFREE, (ch + 1) * NFREE)
                        ps_q = psmm.tile([P, NFREE], F32, name="ps_q", tag="mm")
                        for kt in range(KT):
                            nc.tensor.matmul(ps_q, wq_t[:, kt, ot * P:(ot + 1) * P],
                                             xT_t[:, kt, sl], start=(kt == 0), stop=(kt == KT - 1))
                        nc.vector.tensor_scalar_add(qT_t[:, ot, sl], ps_q, bq_t[:, ot:ot + 1])
                        ps_k = psmm.tile([P, NFREE], F32, name="ps_k", tag="mm")
                        for kt in range(KT):
                            nc.tensor.matmul(ps_k, wk_t[:, kt, ot * P:(ot + 1) * P],
                                             xT_t[:, kt, sl], start=(kt == 0), stop=(kt == KT - 1))
                        nc.vector.tensor_scalar_add(kT_t[:, ot, sl], ps_k, bk_t[:, ot:ot + 1])
                # v[s, o]: lhsT = xT[:, kt, s_blk], rhs = wvT[:, kt, o_chunk]
                for st in range(RT):
                    for ch in range(NCH):
                        sl = slice(ch * NFREE, (ch + 1) * NFREE)
                        ps_v = psmm.tile([P, NFREE], F32, name="ps_v", tag="mm")
                        for kt in range(KT):
                            nc.tensor.matmul(ps_v, xT_t[:, kt, st * P:(st + 1) * P],
                                             wv_t[:, kt, sl], start=(kt == 0), stop=(kt == KT - 1))
                        nc.vector.tensor_tensor(out=v_t[:, st, sl], in0=ps_v, in1=bv_t[:, sl],
                                                op=mybir.AluOpType.add)

                # ---- attention, software-pipelined over row blocks ----
                # Emit scores+softmax for block r, then the PV half of block
                # r-1: the PE's static stream then runs scores(r+1) while the
                # DVE/ACT softmax of block r is still in flight, instead of
                # stalling ~3.5us per block waiting for e(r).
                def emit_scores_softmax(r):
                    sc_t = soft.tile([P, S], F32, name="sc_t", tag="sc")
                    for ch in range(NCH):
                        sl = slice(ch * NFREE, (ch + 1) * NFREE)
                        ps_s = psmm.tile([P, NFREE], F32, name="ps_s", tag="mm")
                        for ot in range(KT):
                            nc.tensor.matmul(ps_s, qT_t[:, ot, r * P:(r + 1) * P],
                                             kT_t[:, ot, sl], start=(ot == 0), stop=(ot == KT - 1))
                        nc.vector.tensor_tensor(out=sc_t[:, sl], in0=ps_s, in1=mkb_t[:, sl],
                                                op=mybir.AluOpType.add)

                    negmax = stats.tile([P, 1], F32, name="negmax", tag="negmax")
                    nc.vector.reduce_max(out=negmax, in_=sc_t, axis=mybir.AxisListType.X, negate=True)
                    e_t = soft.tile([P, S], BF16, name="e_t", tag="e")
                    rowsum = stats.tile([P, 1], F32, name="rowsum", tag="rowsum")
                    nc.scalar.activation(out=e_t, in_=sc_t, func=mybir.ActivationFunctionType.Exp,
                                         bias=negmax, scale=1.0, accum_out=rowsum)
                    recip = stats.tile([P, 1], F32, name="recip", tag="recip")
                    nc.vector.reciprocal(out=recip, in_=rowsum)

                    # attention-weights output: P = e * recip
                    p_t = soft.tile([P, S], F32, name="p_t", tag="p")
                    nc.vector.tensor_scalar_mul(p_t, e_t, recip)
                    nc.sync.dma_start(out=attw[b, r * P:(r + 1) * P, :], in_=p_t)
                    return e_t, recip

                def emit_pv(r, e_t, recip):
                    # transpose e on the PE: eT[j, i] tiles
                    eT_t = soft.tile([P, RT, P], BF16, name="eT_t", tag="eT")
                    for jt in range(RT):
                        ps_t = pstr.tile([P, P], BF16, name="ps_t", tag="tr")
                        nc.tensor.transpose(ps_t, e_t[:, jt * P:(jt + 1) * P], ident)
                        nc.scalar.activation(out=eT_t[:, jt], in_=ps_t,
                                             func=mybir.ActivationFunctionType.Copy)

                    # att[i, h] = sum_j e[i, j] v[j, h], normalized by recip
                    at_t = soft.tile([P, H], F32, name="at_t", tag="at")
                    for ch in range(NCH):
                        sl = slice(ch * NFREE, (ch + 1) * NFREE)
                        ps_a = psmm.tile([P, NFREE], F32, name="ps_a", tag="mm")
                        for jt in range(RT):
                            nc.tensor.matmul(ps_a, eT_t[:, jt], v_t[:, jt, sl],
                                             start=(jt == 0), stop=(jt == RT - 1))
                        nc.vector.tensor_scalar_mul(at_t[:, sl], ps_a, recip)
                    nc.sync.dma_start(out=att[b, r * P:(r + 1) * P, :], in_=at_t)

                prev = None
                for r in range(RT):
                    cur = emit_scores_softmax(r)
                    if prev is not None:
                        emit_pv(r - 1, *prev)
                    prev = cur
                emit_pv(RT - 1, *prev)

    nc.finalize()
    _BUILD_CACHE["nc"] = nc
    return nc


def _bf16(x):
    return np.ascontiguousarray(x.astype(ml_dtypes.bfloat16))


def kernel(input, mask, Wq, bq, Wk, bk, Wv, bv):
    input = np.asarray(input, dtype=np.float32)
    mask = np.asarray(mask)
    scale = np.float32(1.0 / np.sqrt(H))

    wqT = _bf16(Wq.T * scale)
    wkT = _bf16(np.asarray(Wk).T)
    wvT = _bf16(np.asarray(Wv).T)
    bqp = np.ascontiguousarray((np.asarray(bq, dtype=np.float32) * scale).reshape(KT, P).T)
    bkp = np.ascontiguousarray(np.asarray(bk, dtype=np.float32).reshape(KT, P).T)
    bvr = np.ascontiguousarray(np.broadcast_to(np.asarray(bv, dtype=np.float32), (P, H)))

    in_maps = []
    for c in range(NCORES):
        xb = input[c * B_LOC:(c + 1) * B_LOC]                       # [B_LOC, S, H]
        xT = _bf16(xb.transpose(0, 2, 1))                           # [B_LOC, H, S]
        mb = np.where(mask[c * B_LOC:(c + 1) * B_LOC, 0, 0, :] == 0,
                      np.float32(-1e9), np.float32(0.0))            # [B_LOC, S]
        mkb = np.ascontiguousarray(np.broadcast_to(mb[:, None, :], (B_LOC, P, S)),
                                   dtype=np.float32)
        in_maps.append({
            "xT": xT, "wqT": wqT, "wkT": wkT, "wvT": wvT,
            "bqp": bqp, "bkp": bkp, "bvr": bvr, "mkb": mkb,
        })

    nc = build()
    res = run_bass_kernel_spmd(nc, in_maps, core_ids=list(range(NCORES)))
    att = np.concatenate([res.results[c]["att"] for c in range(NCORES)], axis=0)
    attw = np.concatenate([res.results[c]["attw"] for c in range(NCORES)], axis=0)
    return att, attw
